# revision 1
# baseline (speedup 1.0000x reference)
"""MoE (Gemma-style 8-expert top-2) Trainium2 kernel — expert-quad / I-quarter.

Strategy (8 NeuronCores):
  - Host: merge duplicate (token, expert) picks, build per-expert token lists.
    Sort experts by count desc; group A = ranks {0,2,4,6}, group B = ranks
    {1,3,5,7}.  Cores 0-3 serve group A (one I-quarter each), cores 4-7 group
    B.  Every core processes ALL tokens of its group's 4 experts on 1/4 of
    the intermediate dim.  SPMD-common slice widths W[s] = ceil8(max over the
    two groups of the rank-s count) — near-ideal load balance (sum(W) ~ total
    pairs / 2) vs one-expert-per-core which pays 8*max(count).
  - Device (per core): phase 1 runs slice-serial (slice s's x streams in
    during slice s-1's matmuls, so the critical early bytes are just
    wg0 + x_slice0); phase 2 emits fp16 partial down-projections.
  - Host: combine — sum the 4 I-quarter partials per group, then
    out[t] += route[t,e] * y[:, pos].T (exact reference scatter-add).

Falls back to the one-expert-per-core program when a slice exceeds 512.
"""

import numpy as np

import concourse.bass as bass
import concourse.mybir as mybir
import concourse.tile as tile
from concourse import bacc


def _install_ntff_hook_shim():
    import sys
    import types

    try:
        import antenv

        try:
            from antenv import axon_hooks  # noqa: F401

            return
        except ImportError:
            pass
        mod = types.ModuleType("antenv.axon_hooks")
        mod._hook = None
        mod.set_axon_ntff_profile_hook = lambda h: setattr(mod, "_hook", h)
        mod.get_axon_ntff_profile_hook = lambda: mod._hook
        sys.modules["antenv.axon_hooks"] = mod
        antenv.axon_hooks = mod
        import os

        so_path = "/opt/axon/libaxon_pjrt.so"
        if os.path.exists(so_path):
            from trn_agent_boot.trn_boot import _ntff_profile_via_ctypes

            mod._hook = _ntff_profile_via_ctypes(so_path)
    except Exception:
        pass


_install_ntff_hook_shim()

from concourse.bass_utils import run_bass_kernel_spmd

H = 2048
I = 4096
E = 8
P = 128
KH = H // P  # 16 contraction chunks for gate/up
IQ = I // 4  # per-core I slice (1024)
MI4 = IQ // P  # 8 output tiles of I/4
KI4 = IQ // P  # 8 contraction chunks for down (per I-quarter)
MH = H // P  # 16 output tiles of H
MI = I // P  # 32 output tiles of I (fallback single-expert program)
KI = I // P  # 32 contraction chunks for down (fallback)
NS = 4  # slices (experts) per core
F32 = mybir.dt.float32
F16 = mybir.dt.float16

LAST_RESULTS = None

_PROGRAM_CACHE: dict[tuple, "bass.Bass"] = {}


def _build_program_quad(W) -> "bass.Bass":
    """Bass program for one core: I/4 slice of a 4-expert MLP.

    Slice s (W[s] tokens, expert s's weights) occupies output columns
    [off[s], off[s]+W[s]) where off = cumsum(W).
    """
    W = tuple(W)
    assert all(w % 8 == 0 and 8 <= w <= 512 for w in W)
    off = [0]
    for w in W:
        off.append(off[-1] + w)
    CT = off[-1]

    nc = bacc.Bacc("TRN2", target_bir_lowering=False)

    XG = KH // 4
    xTs = [
        nc.dram_tensor(f"xT{s}", [XG, P, 4 * W[s]], F16, kind="ExternalInput")
        for s in range(NS)
    ]
    Wgs = [
        nc.dram_tensor(f"Wg{s}", [MI4, P, KH * P], F16, kind="ExternalInput")
        for s in range(NS)
    ]
    Wus = [
        nc.dram_tensor(f"Wu{s}", [MI4, P, KH * P], F16, kind="ExternalInput")
        for s in range(NS)
    ]
    Wds = [
        nc.dram_tensor(f"Wd{s}", [MH, P, KI4 * P], F16, kind="ExternalInput")
        for s in range(NS)
    ]
    yT = nc.dram_tensor("yT", [H, CT], F16, kind="ExternalOutput")

    xT_r = [t.ap() for t in xTs]
    Wg_a = [t.ap() for t in Wgs]
    Wu_a = [t.ap() for t in Wus]
    Wd_a = [t.ap() for t in Wds]
    yT_r = yT.rearrange("(m p) c -> p m c", p=P)  # [128, 16, CT]

    gelu = mybir.ActivationFunctionType.Gelu_apprx_tanh

    with tile.TileContext(nc) as tc:
        with (
            tc.tile_pool(name="xpool", bufs=1) as xpool,
            tc.tile_pool(name="hpool", bufs=1) as hpool,
            tc.tile_pool(name="wpool", bufs=3) as wpool,
            tc.tile_pool(name="wdpool", bufs=20) as wdpool,
            tc.tile_pool(name="tpool", bufs=3) as tpool,
            tc.tile_pool(name="warm", bufs=1) as warm_pool,
            tc.tile_pool(name="psum2", bufs=2, space="PSUM") as psum_pool,
        ):
            # --- PE warm-up: dummy matmuls cover BOTH the ~3.4us HAM clock
            # ramp AND the ~6us until the first weight/x bytes arrive
            # (~300 GB/s fair-shared early, chip-HBM-limited).
            wz = warm_pool.tile([P, P], F16)
            xz = warm_pool.tile([P, W[0]], F16)
            nc.vector.memset(wz, 0.0)
            nc.vector.memset(xz, 0.0)
            psum_w = psum_pool.tile([P, W[0]], F32, tag="g")
            for _ in range(13):
                nc.tensor.matmul(psum_w, wz, xz, start=True, stop=True)

            xsb = [
                xpool.tile([P, KH, W[s]], F16, tag=f"x{s}", name=f"xsb{s}")
                for s in range(NS)
            ]
            hsb = [
                hpool.tile([P, KI4, W[s]], F16, tag=f"h{s}", name=f"hsb{s}")
                for s in range(NS)
            ]

            def load_w(dram_ap, t, tag, pool=wpool, cols=KH * P, splits=1):
                wt = pool.tile([P, cols], F16, tag=tag, name=f"w_{tag}_{t}")
                step = cols // splits
                for sp in range(splits):
                    nc.sync.dma_start(
                        out=wt[:, sp * step : (sp + 1) * step],
                        in_=dram_ap[t, :, sp * step : (sp + 1) * step],
                    )
                return wt.rearrange("p (k i) -> p k i", i=P)

            def load_x(s, g):
                nc.sync.dma_start(
                    out=xsb[s][:, 4 * g : 4 * (g + 1), :], in_=xT_r[s][g]
                )

            # Early DMA order: first pass's m0 weights + its x only (the
            # ~2.5MB critical set rides the ~300 GB/s HBM-limited early
            # window); later passes' x is staged during earlier passes.
            p1_order = list(range(NS))
            s0 = p1_order[0]
            wg_t0 = load_w(Wg_a[s0], 0, "wg", splits=2)
            for g in range(XG):
                load_x(s0, g)
            wu_t0 = load_w(Wu_a[s0], 0, "wu")
            wg_t1 = load_w(Wg_a[s0], 1, "wg")
            wu_t1 = load_w(Wu_a[s0], 1, "wu")

            # ---- Phase 1: gateT/upT -> hT, slice-serial
            for si, s in enumerate(p1_order):
                for m in range(MI4):
                    if si == 0 and m == 0:
                        wg_t, wu_t = wg_t0, wu_t0
                    elif si == 0 and m == 1:
                        wg_t, wu_t = wg_t1, wu_t1
                    else:
                        wg_t = load_w(Wg_a[s], m, "wg")
                        wu_t = load_w(Wu_a[s], m, "wu")
                    if m == 2 and si < NS - 1:
                        # next pass's x streams during this pass
                        for g in range(XG):
                            load_x(p1_order[si + 1], g)

                    psum_g = psum_pool.tile([P, W[s]], F32, tag="g")
                    psum_u = psum_pool.tile([P, W[s]], F32, tag="u")
                    for k in range(KH):
                        nc.tensor.matmul(
                            psum_g, wg_t[:, k, :], xsb[s][:, k, :],
                            start=(k == 0), stop=(k == KH - 1),
                        )
                    for k in range(KH):
                        nc.tensor.matmul(
                            psum_u, wu_t[:, k, :], xsb[s][:, k, :],
                            start=(k == 0), stop=(k == KH - 1),
                        )
                    tg = tpool.tile([P, W[s]], F32, tag="gelu")
                    nc.scalar.activation(tg, psum_g, gelu)
                    nc.vector.tensor_mul(hsb[s][:, m, :], tg, psum_u)

            # ---- Phase 2: downT partials -> yT (fp16); the last slice of
            # the last H-tile is split so little copy+DMA trails the end
            ptags = ["g", "u"]
            pidx = 0
            for m2 in range(MH):
                wd_ts = [
                    load_w(Wd_a[s], m2, "wd", pool=wdpool, cols=KI4 * P)
                    for s in range(NS)
                ]
                pieces = [(s, 0, W[s]) for s in range(NS)]
                if m2 == MH - 1:
                    # split the last slice so only a small copy+DMA trails
                    # the final matmul
                    bh = (W[NS - 1] // 2) // 8 * 8
                    pieces = pieces[:-1] + [
                        (NS - 1, 0, bh),
                        (NS - 1, bh, W[NS - 1]),
                    ]
                # one [P, CT] fp16 staging tile per m2: a single y DMA with
                # ~4KB elements instead of 4 small ones with ~1KB elements
                # (few big descriptors keep the wd stream from starving)
                ysb = tpool.tile([P, CT], F16, tag="ystage", bufs=2)
                for s, c0, c1 in pieces:
                    psum_d = psum_pool.tile([P, c1 - c0], F32, tag=ptags[pidx % 2])
                    pidx += 1
                    for k2 in range(KI4):
                        nc.tensor.matmul(
                            psum_d, wd_ts[s][:, k2, :], hsb[s][:, k2, c0:c1],
                            start=(k2 == 0), stop=(k2 == KI4 - 1),
                        )
                    nc.vector.tensor_copy(
                        ysb[:, off[s] + c0 : off[s] + c1], psum_d
                    )
                    if m2 == MH - 1 and s == NS - 2:
                        # flush the first 3 slices while slice 3 computes
                        nc.sync.dma_start(
                            out=yT_r[:, m2, 0 : off[NS - 1]],
                            in_=ysb[:, 0 : off[NS - 1]],
                        )
                if m2 < MH - 1:
                    nc.sync.dma_start(out=yT_r[:, m2, :], in_=ysb)
                else:
                    nc.sync.dma_start(
                        out=yT_r[:, m2, off[NS - 1] : CT],
                        in_=ysb[:, off[NS - 1] : CT],
                    )

    nc.compile()
    return nc


def _get_program_quad(W) -> "bass.Bass":
    key = ("quad",) + tuple(W)
    if key not in _PROGRAM_CACHE:
        _PROGRAM_CACHE[key] = _build_program_quad(W)
    return _PROGRAM_CACHE[key]


def _prep_w_gu_q(w):  # [H, IQ] f32 -> [MI4, P, KH*P] fp16
    return np.ascontiguousarray(
        w.astype(np.float16).reshape(KH, P, MI4, P).transpose(2, 1, 0, 3)
    ).reshape(MI4, P, KH * P)


def _prep_w_d_q(w):  # [IQ, H] f32 -> [MH, P, KI4*P] fp16
    return np.ascontiguousarray(
        w.astype(np.float16).reshape(KI4, P, MH, P).transpose(2, 1, 0, 3)
    ).reshape(MH, P, KI4 * P)


def _ceil8(n):
    return max(8, -(-n // 8) * 8)


def kernel(x, selected_experts, routing_weights, Wg, Wu, Wd):
    global LAST_RESULTS
    x = np.asarray(x, dtype=np.float32)
    se = np.asarray(selected_experts).astype(np.int64)
    rw = np.asarray(routing_weights).astype(np.float32)
    Wg = np.asarray(Wg, dtype=np.float32)
    Wu = np.asarray(Wu, dtype=np.float32)
    Wd = np.asarray(Wd, dtype=np.float32)

    T, K = se.shape
    assert x.shape == (T, H) and Wg.shape == (E, H, I) and Wd.shape == (E, I, H)

    # Dense route matrix, identical to the reference's scatter-add (merges
    # duplicate expert picks within a token by summing their weights).
    flat_t = np.repeat(np.arange(T), K)
    flat_e = se.ravel()
    route = np.zeros((T, E), np.float32)
    np.add.at(route, (flat_t, flat_e), rw.ravel())
    present = np.zeros((T, E), bool)
    present[flat_t, flat_e] = True

    idx_lists = [np.nonzero(present[:, e])[0] for e in range(E)]
    counts = np.array([len(ix) for ix in idx_lists])

    order = np.argsort(-counts, kind="stable")
    groups = [order[0::2], order[1::2]]  # ranks {0,2,4,6} and {1,3,5,7}
    W = [
        _ceil8(int(max(counts[groups[0][s]], counts[groups[1][s]])))
        for s in range(NS)
    ]
    if max(W) <= 512:
        return _kernel_quad(x, route, idx_lists, groups, W, Wg, Wu, Wd)
    return _kernel_single(x, route, present, idx_lists, Wg, Wu, Wd)


def _kernel_quad(x, route, idx_lists, groups, W, Wg, Wu, Wd):
    global LAST_RESULTS
    T = x.shape[0]
    off = np.concatenate([[0], np.cumsum(W)])

    nc = _get_program_quad(W)
    xhalf = x.astype(np.float16)

    def pack_x(ix, C):
        # [H, C] -> [XG=4, P, 4*C]: partition-major 4-chunk groups
        # (multi-KB contiguous DMA elements per partition)
        xt = np.zeros((H, C), np.float16)
        if len(ix):
            xt[:, : len(ix)] = xhalf[ix].T
        return np.ascontiguousarray(
            xt.reshape(4, 4, P, C).transpose(0, 2, 1, 3)
        ).reshape(4, P, 4 * C)

    in_maps = []
    for g in range(2):
        exps = [int(e) for e in groups[g]]
        xpacks = {
            f"xT{s}": pack_x(idx_lists[exps[s]], W[s]) for s in range(NS)
        }
        for q in range(4):
            sl = slice(q * IQ, (q + 1) * IQ)
            m = dict(xpacks)
            for s in range(NS):
                m[f"Wg{s}"] = _prep_w_gu_q(Wg[exps[s]][:, sl])
                m[f"Wu{s}"] = _prep_w_gu_q(Wu[exps[s]][:, sl])
                m[f"Wd{s}"] = _prep_w_d_q(Wd[exps[s]][sl, :])
            in_maps.append(m)
    res = run_bass_kernel_spmd(nc, in_maps, core_ids=list(range(E)))
    LAST_RESULTS = res

    out = np.zeros((T, H), np.float32)
    for g in range(2):
        exps = [int(e) for e in groups[g]]
        ysum = sum(
            res.results[4 * g + q]["yT"].astype(np.float32) for q in range(4)
        )  # [H, CT]
        for s in range(NS):
            ix = idx_lists[exps[s]]
            if len(ix):
                out[ix] += (
                    route[ix, exps[s]][:, None]
                    * ysum[:, off[s] : off[s] + len(ix)].T
                )
    return out


def _build_program_single(C: int) -> "bass.Bass":
    """Bass program for one core: expert MLP on C tokens (transposed layout)."""
    assert C % 8 == 0 and 256 <= C <= 512

    nc = bacc.Bacc("TRN2", target_bir_lowering=False)

    # Host-prepacked inputs: each [t, :, :] slab is one SBUF tile, contiguous.
    # x is packed partition-major in 4-chunk groups: [g, p, j, c] so each DMA
    # element is 4*C*2 = 4KB contiguous per partition (the 16 parallel DMA
    # engines run ~4x faster on 4KB elements than on 1KB rows).
    XG = KH // 4
    xT = nc.dram_tensor("xT", [XG, P, 4 * C], F16, kind="ExternalInput")
    Wg = nc.dram_tensor("Wg", [MI, P, KH * P], F16, kind="ExternalInput")
    Wu = nc.dram_tensor("Wu", [MI, P, KH * P], F16, kind="ExternalInput")
    Wd = nc.dram_tensor("Wd", [MH, P, KI * P], F16, kind="ExternalInput")
    yT = nc.dram_tensor("yT", [H, C], F32, kind="ExternalOutput")

    xT_r = xT.ap()  # [XG, 128, 4*C]
    yT_r = yT.rearrange("(m p) c -> p m c", p=P)  # [128, 16, C]
    Wg_a, Wu_a, Wd_a = Wg.ap(), Wu.ap(), Wd.ap()

    gelu = mybir.ActivationFunctionType.Gelu_apprx_tanh

    with tile.TileContext(nc) as tc:
        with (
            tc.tile_pool(name="xpool", bufs=1) as xpool,
            tc.tile_pool(name="hpool", bufs=1) as hpool,
            tc.tile_pool(name="wpool", bufs=6) as wpool,
            tc.tile_pool(name="tpool", bufs=3) as tpool,
            tc.tile_pool(name="warm", bufs=1) as warm_pool,
            tc.tile_pool(name="psum", bufs=2) as _psum_unused,  # keep name stable
            tc.tile_pool(name="psum2", bufs=2, space="PSUM") as psum_pool,
            tc.tile_pool(name="psumw", bufs=1, space="PSUM") as psum_warm,
        ):
            # --- PE warm-up: dummy matmuls over zeros while first DMAs land.
            # Must cover BOTH the ~3.4us HAM clock ramp AND the ~6us until
            # the first weight/x bytes arrive (~300 GB/s fair-shared across
            # the in-flight descriptor window) — a >1us PE gap after warmup
            # would re-throttle the clock and cost more than it saves.
            wz = warm_pool.tile([P, P], F16)
            xz = warm_pool.tile([P, C], F16)
            nc.vector.memset(wz, 0.0)
            nc.vector.memset(xz, 0.0)
            psum_w = psum_warm.tile([P, C], F32, tag="warm")
            for _ in range(13):
                nc.tensor.matmul(psum_w, wz, xz, start=True, stop=True)

            # x resident in SBUF: [128, 16, C] fp16
            xsb = xpool.tile([P, KH, C], F16)

            # h resident in SBUF: [128, 32, C] fp16
            hsb = hpool.tile([P, KI, C], F16)

            def load_w(dram_ap, t, tag, splits=1):
                wt = wpool.tile([P, KH * P], F16, tag=tag, name=f"w_{tag}_{t}")
                step = (KH * P) // splits
                for s in range(splits):
                    nc.sync.dma_start(
                        out=wt[:, s * step : (s + 1) * step],
                        in_=dram_ap[t, :, s * step : (s + 1) * step],
                    )
                return wt.rearrange("p (k i) -> p k i", i=P)

            def load_x(g):
                nc.sync.dma_start(out=xsb[:, 4 * g : 4 * (g + 1), :], in_=xT_r[g])

            # Early DMAs fair-share ~300 GB/s across 16 parallel engines, so
            # what matters is bytes in flight vs first-need: weights for m0
            # and the x groups m0's k-loop consumes first, then m1 weights.
            wg_t0 = load_w(Wg_a, 0, "wg", splits=2)
            load_x(0)
            load_x(1)
            wu_t0 = load_w(Wu_a, 0, "wu", splits=2)
            load_x(2)
            load_x(3)
            wg_t1 = load_w(Wg_a, 1, "wg")
            wu_t1 = load_w(Wu_a, 1, "wu")

            # ---- Phase 1: gateT/upT -> hT, one I-tile (128 rows) at a time
            for m in range(MI):
                if m == 0:
                    wg_t, wu_t = wg_t0, wu_t0
                elif m == 1:
                    wg_t, wu_t = wg_t1, wu_t1
                else:
                    wg_t = load_w(Wg_a, m, "wg")
                    wu_t = load_w(Wu_a, m, "wu")

                psum_g = psum_pool.tile([P, C], F32, tag="g")
                psum_u = psum_pool.tile([P, C], F32, tag="u")
                for k in range(KH):
                    nc.tensor.matmul(
                        psum_g,
                        wg_t[:, k, :],
                        xsb[:, k, :],
                        start=(k == 0),
                        stop=(k == KH - 1),
                    )
                for k in range(KH):
                    nc.tensor.matmul(
                        psum_u,
                        wu_t[:, k, :],
                        xsb[:, k, :],
                        start=(k == 0),
                        stop=(k == KH - 1),
                    )
                tg = tpool.tile([P, C], F32, tag="gelu")
                nc.scalar.activation(tg, psum_g, gelu)
                nc.vector.tensor_mul(hsb[:, m, :], tg, psum_u)

            # ---- Phase 2: downT -> yT, one H-tile (128 rows) at a time
            for m2 in range(MH):
                wd_t = wpool.tile([P, KI * P], F16, tag="wd", name=f"w_wd_{m2}")
                nc.sync.dma_start(out=wd_t, in_=Wd_a[m2])
                wd_v = wd_t.rearrange("p (k i) -> p k i", i=P)
                if m2 < MH - 2:
                    pieces = [(0, C)]
                elif m2 == MH - 2:
                    pieces = [(0, C // 2), (C // 2, C)]
                else:
                    # last tile: narrow column pieces so only ~C/4 columns of
                    # copy+DMA remain exposed after the final matmul
                    q = -(-C // 32) * 8  # ceil(C/4) rounded up to mult of 8
                    pieces = [(i * q, min((i + 1) * q, C)) for i in range(4)]
                    pieces = [(a, b) for a, b in pieces if b > a]
                ptags = ["d", "g"]
                for pi, (c0, c1) in enumerate(pieces):
                    psum_d = psum_pool.tile([P, c1 - c0], F32, tag=ptags[pi % 2])
                    for k2 in range(KI):
                        nc.tensor.matmul(
                            psum_d,
                            wd_v[:, k2, :],
                            hsb[:, k2, c0:c1],
                            start=(k2 == 0),
                            stop=(k2 == KI - 1),
                        )
                    ysb = tpool.tile([P, c1 - c0], F32, tag="y")
                    nc.vector.tensor_copy(ysb, psum_d)
                    nc.sync.dma_start(out=yT_r[:, m2, c0:c1], in_=ysb)

    nc.compile()
    return nc


def _prep_w_gu(w):  # [H, I] f32 -> [MI, P, KH*P] 16-bit, per-tile contiguous
    return np.ascontiguousarray(
        w.astype(np.float16).reshape(KH, P, MI, P).transpose(2, 1, 0, 3)
    ).reshape(MI, P, KH * P)


def _prep_w_d(w):  # [I, H] f32 -> [MH, P, KI*P] 16-bit
    return np.ascontiguousarray(
        w.astype(np.float16).reshape(KI, P, MH, P).transpose(2, 1, 0, 3)
    ).reshape(MH, P, KI * P)



def _get_program_single(C: int) -> "bass.Bass":
    key = ("single", C)
    if key not in _PROGRAM_CACHE:
        _PROGRAM_CACHE[key] = _build_program_single(C)
    return _PROGRAM_CACHE[key]


def _kernel_single(x, route, present, idx_lists, Wg, Wu, Wd):
    """Fallback: one expert per core, capacity-chunked (handles any skew)."""
    global LAST_RESULTS
    T = x.shape[0]
    chunked = [
        [ix[s : s + 512] for s in range(0, max(len(ix), 1), 512)]
        for ix in idx_lists
    ]
    n_pass = max(len(ch) for ch in chunked)

    out = np.zeros((T, H), np.float32)
    for p in range(n_pass):
        parts = [ch[p] if p < len(ch) else np.empty(0, np.int64) for ch in chunked]
        max_count = max(len(ix) for ix in parts)
        C = max(256, min(512, -(-max(max_count, 1) // 8) * 8))
        nc = _get_program_single(C)
        in_maps = []
        for e in range(E):
            ix = parts[e]
            xT_e = np.zeros((H, C), np.float16)
            if len(ix):
                xT_e[:, : len(ix)] = x[ix].T.astype(np.float16)
            # [H, C] -> [XG=4, P, 4*C]: partition-major 4-chunk groups
            xT_e = np.ascontiguousarray(
                xT_e.reshape(4, 4, P, C).transpose(0, 2, 1, 3)
            ).reshape(4, P, 4 * C)
            in_maps.append(
                {
                    "xT": xT_e,
                    "Wg": _prep_w_gu(Wg[e]),
                    "Wu": _prep_w_gu(Wu[e]),
                    "Wd": _prep_w_d(Wd[e]),
                }
            )
        res = run_bass_kernel_spmd(nc, in_maps, core_ids=list(range(E)))
        LAST_RESULTS = res
        for e in range(E):
            ix = parts[e]
            if len(ix) == 0:
                continue
            yT_e = res.results[e]["yT"]  # [H, C]
            out[ix] += route[ix, e][:, None] * yT_e[:, : len(ix)].T
    return out

